# revision 8
# baseline (speedup 1.0000x reference)
"""Trainium2 Bass kernel for nn_Decoder_Cross_Projector.

Computation: kv = node @ W + b  -> split K/V caches -> rotary-rotate K by
mass sin/cos -> [2, B, H, N, KEY].

Sharding (8 cores, tensor-parallel on the head axis): core i owns k-heads
[16i,16i+16) and v-heads [16i,16i+16), i.e. a [1024, 2048] column slice of W.
`node` is replicated (transposed on host so the contraction dim lands on SBUF
partitions). Each core runs an identical program on its slice; outputs are
re-assembled host-side. No collectives.

Per-core device program (Tile framework):
  - W slice + broadcast bias resident in SBUF; node^T streamed per 128-token
    block; fp32r (fp22-multiply, fp32-accumulate) matmuls at full PE rate:
    64 token blocks x 4 psum tiles x 8 K-chunks = 2048 matmuls of
    [128,128]^T @ [128,512].
  - K-head psum tiles get bias + rotary on DVE (sin/cos built on ACT from a
    range-reduced angle; DVE `mod` does the reduction).
  - Results DMA straight to a [32, 8192, 64] per-core output layout.
"""

import math
import os

import numpy as np

import concourse.bass as bass
import concourse.tile as tile
from concourse import mybir
from concourse.bass_utils import run_bass_kernel_spmd
from concourse.tile import ScopedClock
from bass_rust import VectorClock, SyncInfo
from concourse.tile_sem_assignment import N_PROCS

f32 = mybir.dt.float32
f32r = mybir.dt.float32r

# ---------------------------------------------------------------------------
# Workarounds for this walrus build: it encodes at most ONE semaphore wait
# per instruction ("Too many sync wait commands" in setupSyncWait).
# (1) Replace TileContext's end-of-context drain (which carries one wait per
#     logical proc) with a chain of single-wait drains.
# (2) After tracing, hoist extra waits from any multi-wait instruction onto
#     InstNoOp carriers inserted immediately before it on the same engine.
# Both preserve semantics exactly: waits execute on the same engine stream,
# in the same order, before the guarded instruction.
# ---------------------------------------------------------------------------


def _drain_and_barrier_chunked(self, tick_clock, wait_clock):
    gc = tick_clock.global_clock
    prev = VectorClock()
    emitted = False
    for p in range(N_PROCS):
        if not gc[p]:
            continue
        partial = prev.copy()
        partial.require_at_least(p, gc[p])
        inst = self.nc.sync.drain()
        wait_clock.add_sem_waits(
            inst.ins, ScopedClock({None: partial}), ScopedClock({None: prev})
        )
        prev = partial
        emitted = True
    if not emitted:
        self.nc.sync.drain()
    self.nc.all_engine_barrier()
    assert self.sems is not None
    popped = self.nc._tile_sem_poison_stack.pop()
    assert popped is self._sem_poison
    self.nc.clear_and_free_semaphores(list(self.sems.allocated().values()))
    self.nc.all_engine_barrier()


tile.TileContext._drain_and_barrier = _drain_and_barrier_chunked

_DMA_INSTS = {"InstDMACopy", "InstDMA", "InstDmaTransposeAnt"}


def _split_multi_waits(nc):
    n_split = 0
    for f in nc.m.functions:
        for bb in f.blocks:
            insts = bb.instructions
            out = []
            changed = False
            for inst in insts:
                si = inst.sync_info
                if si is not None and len(si.on_wait) > 1:
                    # Keep a DMA-queue flow-control wait (DMAHW*/DMASW*) on
                    # the instruction itself; hoist the rest onto carriers.
                    waits = sorted(
                        si.on_wait,
                        key=lambda w: ("DMAHW" in w.ant_name
                                       or "DMASW" in w.ant_name)
                        if type(inst).__name__ in _DMA_INSTS else False,
                    )
                    for w in waits[:-1]:
                        nop = mybir.InstNoOp(
                            name=f"{inst.name}_waitc{n_split}", ins=[], outs=[]
                        )
                        nop.engine = inst.engine
                        nop.sync_info = SyncInfo(on_wait=[w], on_update=[])
                        out.append(nop)
                        n_split += 1
                    inst.sync_info = SyncInfo(
                        on_wait=[waits[-1]], on_update=list(si.on_update)
                    )
                    changed = True
                out.append(inst)
            if changed:
                bb.instructions = out
    return n_split


# ---------------------------------------------------------------------------
# Problem constants (hardcoded per the contract)
# ---------------------------------------------------------------------------
N_CORES = 8
B, SEQ, HIDDEN = 4, 2048, 1024
NUM_LAYERS, REL_SIZE, KEY = 8, 16, 64
HALF = KEY // 2  # 32
H = REL_SIZE * NUM_LAYERS  # 128 heads per cache
T = B * SEQ  # 8192 tokens
HPC = 2 * H // N_CORES  # 32 head-slots per core (16 K + 16 V)
FPC = HPC * KEY  # 2048 output features per core
KC = HIDDEN // 128  # 8 contraction chunks
NF = FPC // 512  # 4 psum tiles per token block
PI = math.pi

LAST_EXEC_TIME_NS = None


def build_nc(n_mblk=T // 128, split_waits=True):
    nc = bass.Bass()
    nodeT = nc.dram_tensor("nodeT", [HIDDEN, T], f32r, kind="ExternalInput")
    w = nc.dram_tensor("w", [HIDDEN, FPC], f32r, kind="ExternalInput")
    biasb = nc.dram_tensor("biasb", [128, FPC], f32, kind="ExternalInput")
    massr = nc.dram_tensor("massr", [128, T // 128], f32, kind="ExternalInput")
    invf = nc.dram_tensor("invf", [128, HALF], f32, kind="ExternalInput")
    out = nc.dram_tensor("out", [HPC, T, KEY], f32, kind="ExternalOutput")

    with tile.TileContext(nc) as tc:
        with tc.tile_pool(name="wpool", bufs=1) as wpool, \
             tc.tile_pool(name="cpool", bufs=1) as cpool, \
             tc.tile_pool(name="npool", bufs=3) as npool, \
             tc.tile_pool(name="opool", bufs=6) as opool, \
             tc.tile_pool(name="tpool", bufs=3) as tpool, \
             tc.tile_pool(name="scpool", bufs=3) as scpool, \
             tc.tile_pool(name="pspool", bufs=8, space="PSUM") as pspool:

            w_sb = wpool.tile([128, KC, FPC], f32r)
            nc.sync.dma_start(
                w_sb[:], w[:].rearrange("(kc p) n -> p kc n", p=128))
            biasb_sb = cpool.tile([128, FPC], f32)
            nc.sync.dma_start(biasb_sb[:], biasb[:])
            invf_sb = cpool.tile([128, HALF], f32)
            nc.sync.dma_start(invf_sb[:], invf[:])
            massr_sb = cpool.tile([128, T // 128], f32)
            nc.sync.dma_start(massr_sb[:], massr[:])

            for m in range(n_mblk):
                nt = npool.tile([128, KC, 128], f32r)
                nc.sync.dma_start(
                    nt[:],
                    nodeT[:, m * 128:(m + 1) * 128].rearrange(
                        "(kc p) t -> p kc t", p=128))

                # --- angle + sin/cos for this token block (ACT + DVE) ---
                # HW Sin is only accurate for |x| <= pi; f32->i32 conversion
                # rounds to nearest, so red = ang - 2pi*i32(ang/2pi) lands in
                # [-pi, pi]. cos(ang) = sin(ang + pi/2), reduced the same way.
                mass_col = massr_sb[:, m:m + 1]
                t1 = scpool.tile([128, HALF], f32)  # ang
                nc.vector.tensor_scalar(
                    t1[:], invf_sb[:], mass_col, None, mybir.AluOpType.mult)
                t2 = scpool.tile([128, HALF], f32)  # ang + pi/2
                nc.vector.tensor_scalar(
                    t2[:], t1[:], 0.5 * PI, None, mybir.AluOpType.add)
                q1 = scpool.tile([128, HALF], mybir.dt.int32)
                nc.vector.tensor_scalar(
                    q1[:], t1[:], 1.0 / (2.0 * PI), None, mybir.AluOpType.mult)
                q2 = scpool.tile([128, HALF], mybir.dt.int32)
                nc.vector.tensor_scalar(
                    q2[:], t2[:], 1.0 / (2.0 * PI), None, mybir.AluOpType.mult)
                qf1 = scpool.tile([128, HALF], f32)
                nc.vector.tensor_copy(qf1[:], q1[:])
                qf2 = scpool.tile([128, HALF], f32)
                nc.vector.tensor_copy(qf2[:], q2[:])
                s1 = scpool.tile([128, HALF], f32)
                nc.vector.scalar_tensor_tensor(
                    s1[:], qf1[:], -2.0 * PI, t1[:],
                    mybir.AluOpType.mult, mybir.AluOpType.add)
                s2 = scpool.tile([128, HALF], f32)
                nc.vector.scalar_tensor_tensor(
                    s2[:], qf2[:], -2.0 * PI, t2[:],
                    mybir.AluOpType.mult, mybir.AluOpType.add)
                # fold (mode-agnostic): s > pi -> s -= 2pi
                g1 = scpool.tile([128, HALF], f32)
                nc.vector.tensor_scalar(
                    g1[:], s1[:], PI, None, mybir.AluOpType.is_gt)
                g2 = scpool.tile([128, HALF], f32)
                nc.vector.tensor_scalar(
                    g2[:], s2[:], PI, None, mybir.AluOpType.is_gt)
                red = scpool.tile([128, HALF], f32)
                nc.vector.scalar_tensor_tensor(
                    red[:], g1[:], -2.0 * PI, s1[:],
                    mybir.AluOpType.mult, mybir.AluOpType.add)
                redc = scpool.tile([128, HALF], f32)
                nc.vector.scalar_tensor_tensor(
                    redc[:], g2[:], -2.0 * PI, s2[:],
                    mybir.AluOpType.mult, mybir.AluOpType.add)
                snsn = scpool.tile([128, KEY], f32)  # [0:32]=-sin, [32:64]=+sin
                nc.scalar.activation(
                    snsn[:, 0:HALF], red[:], mybir.ActivationFunctionType.Sin,
                    scale=-1.0)
                nc.scalar.activation(
                    snsn[:, HALF:KEY], red[:],
                    mybir.ActivationFunctionType.Sin)
                cos_t = scpool.tile([128, HALF], f32)
                nc.scalar.activation(
                    cos_t[:], redc[:], mybir.ActivationFunctionType.Sin)

                for fi in range(NF):
                    ps = pspool.tile([128, 512], f32)
                    for kc in range(KC):
                        nc.tensor.matmul(
                            ps[:],
                            lhsT=nt[:, kc, :],
                            rhs=w_sb[:, kc, fi * 512:(fi + 1) * 512],
                            start=(kc == 0), stop=(kc == KC - 1))
                    ob = opool.tile([128, 512], f32)
                    bias_sl = biasb_sb[:, fi * 512:(fi + 1) * 512]
                    if fi < NF // 2:
                        # K heads: bias add then rotary
                        tt = tpool.tile([128, 512], f32)
                        nc.vector.tensor_tensor(
                            tt[:], ps[:], bias_sl, mybir.AluOpType.add)
                        t3 = tt[:].rearrange("p (j h d) -> p j h d", j=8, h=2)
                        o3 = ob[:].rearrange("p (j h d) -> p j h d", j=8, h=2)
                        cosb = cos_t[:].unsqueeze(1).unsqueeze(2).to_broadcast(
                            (128, 8, 2, HALF))
                        nc.vector.tensor_tensor(
                            o3, t3, cosb, mybir.AluOpType.mult)
                        m2 = tpool.tile([128, 512], f32)
                        m23 = m2[:].rearrange("p (j h d) -> p j h d", j=8, h=2)
                        negs = snsn[:, 0:HALF].unsqueeze(1).to_broadcast(
                            (128, 8, HALF))
                        sins = snsn[:, HALF:KEY].unsqueeze(1).to_broadcast(
                            (128, 8, HALF))
                        nc.vector.tensor_tensor(
                            m23[:, :, 0, :], t3[:, :, 1, :], negs,
                            mybir.AluOpType.mult)
                        nc.vector.tensor_tensor(
                            m23[:, :, 1, :], t3[:, :, 0, :], sins,
                            mybir.AluOpType.mult)
                        nc.vector.tensor_tensor(
                            ob[:], ob[:], m2[:], mybir.AluOpType.add)
                    else:
                        # V heads: bias add only
                        nc.vector.tensor_tensor(
                            ob[:], ps[:], bias_sl, mybir.AluOpType.add)
                    dst = out[fi * 8:(fi + 1) * 8,
                              m * 128:(m + 1) * 128, :].transpose([1, 0, 2])
                    nc.sync.dma_start(
                        dst, ob[:].rearrange("p (j d) -> p j d", j=8))

    if split_waits:
        _split_multi_waits(nc)
    return nc


def prep_inputs(node, node_mass, W, b):
    """Host-side layout prep + per-core sharding."""
    node = np.ascontiguousarray(np.asarray(node, dtype=np.float32))
    node_mass = np.ascontiguousarray(np.asarray(node_mass, dtype=np.float32))
    W = np.ascontiguousarray(np.asarray(W, dtype=np.float32))
    b = np.ascontiguousarray(np.asarray(b, dtype=np.float32))

    nodeT = np.ascontiguousarray(node.reshape(T, HIDDEN).T)  # [1024, 8192]
    massr = np.ascontiguousarray(
        node_mass.reshape(T // 128, 128).T)  # [128, 64]
    inv_freq = np.exp(
        -np.log(np.float32(10000.0))
        * np.arange(HALF, dtype=np.float32) / np.float32(HALF)
    ).astype(np.float32)
    invf = np.ascontiguousarray(np.broadcast_to(inv_freq, (128, HALF)))

    in_maps = []
    for i in range(N_CORES):
        k_cols = slice(i * 1024, (i + 1) * 1024)
        v_cols = slice(H * KEY + i * 1024, H * KEY + (i + 1) * 1024)
        wi = np.ascontiguousarray(
            np.concatenate([W[:, k_cols], W[:, v_cols]], axis=1))
        bi = np.concatenate([b[k_cols], b[v_cols]])
        biasb = np.ascontiguousarray(
            np.broadcast_to(bi, (128, FPC)).astype(np.float32))
        in_maps.append({
            "nodeT": nodeT, "w": wi, "biasb": biasb,
            "massr": massr, "invf": invf,
        })
    return in_maps


_NC_CACHE = {}


def kernel(node, node_mass, W, b):
    global LAST_EXEC_TIME_NS
    if "nc" not in _NC_CACHE:
        _NC_CACHE["nc"] = build_nc()
    nc = _NC_CACHE["nc"]

    in_maps = prep_inputs(node, node_mass, W, b)
    res = run_bass_kernel_spmd(nc, in_maps, list(range(N_CORES)),
                               trace=False)
    LAST_EXEC_TIME_NS = res.exec_time_ns

    full = np.empty((2, B, H, SEQ, KEY), dtype=np.float32)
    for i in range(N_CORES):
        oc = res.results[i]["out"].reshape(HPC, B, SEQ, KEY)
        full[0, :, 16 * i:16 * (i + 1)] = oc[:16].transpose(1, 0, 2, 3)
        full[1, :, 16 * i:16 * (i + 1)] = oc[16:].transpose(1, 0, 2, 3)
    return full
